# revision 2
# baseline (speedup 1.0000x reference)
"""GATv2 (3-layer, 4-head) message-passing kernel for Trainium2, 8-core SPMD.

Strategy: nodes sharded contiguously across 8 cores; edges partitioned by
destination; per-layer AllGather of the source-side transform xl = x @ Wl so
each core can gather arbitrary source rows; segment softmax / scatter-add
stay local per destination shard.

V4 design notes (HW-measured costs drove these choices):
- Per-k-tile [P,1]-offset indirect gathers are the only working gather
  mechanism on this runtime (multi-column offsets return garbage,
  InstDMAGatherAnt crashes the device). At ~1.42us of Q7 emission per op
  they are the kernel's critical resource, so everything else is kept off
  the gpsimd engine: selection matrices S (segment-sum) and ST (xr
  segment-broadcast) are precomputed one-hot on the host and DMA-streamed,
  per-layer broadcast constants are host-materialized, and the self-loop
  tile (k=0) is loaded with a plain DMA from the local xl shard instead of
  a gather.
- Segment softmax needs no max-subtraction (logits are O(4)); numerator
  uses sum(ee * xl[src]) directly so the xr term cancels:
      out[dst] = sum_e ee_e * xl[src_e] / denom  (+ bl + gat_bias)
- Edge-phase tensors are bf16 for DVE 2x mode; per-k-tile count is
  variable per chunk (max over cores, baked at compile time).
"""

import sys

sys.path.insert(0, "/opt/trn_rl_repo")

import ml_dtypes
import numpy as np

import concourse.bass as bass
import concourse.bacc as bacc
import concourse.tile as tile
from concourse import mybir
from concourse.bass import IndirectOffsetOnAxis

F32 = mybir.dt.float32
I32 = mybir.dt.int32
BF16 = mybir.dt.bfloat16
AF = mybir.ActivationFunctionType
ALU = mybir.AluOpType
AX = mybir.AxisListType

import os

P = 128
NEG_SLOPE = 0.2
LN_EPS = 1e-5
DENOM_EPS = 1e-30


class Cfg:
    def __init__(self, N=50000, D=128, H=4, L=3, n_cores=8):
        self.N, self.D, self.H, self.L, self.M = N, D, H, L, n_cores
        self.C = D // H
        assert N % n_cores == 0
        self.shard = N // n_cores
        self.chunks = (self.shard + P - 1) // P


# ----------------------------------------------------------------------------
# Host preprocessing: sort neighbor edges by dst, pack per-core chunk/k-tile
# arrays; k=0 is the implicit self-loop tile. One-hot S / ST built on host.
# ----------------------------------------------------------------------------

def preprocess(edge_index, cfg):
    N, M, shard, chunks = cfg.N, cfg.M, cfg.shard, cfg.chunks
    ei = np.asarray(edge_index)
    src = ei[0].astype(np.int64)
    dst = ei[1].astype(np.int64)
    order = np.argsort(dst, kind="stable")
    src_s, dst_s = src[order], dst[order]

    # per-(core, chunk) neighbor edge lists
    counts = np.zeros((M, chunks), dtype=np.int64)
    per_core = []
    for c in range(M):
        lo, hi = np.searchsorted(dst_s, [c * shard, (c + 1) * shard])
        d_loc = dst_s[lo:hi] - c * shard
        s_loc = src_s[lo:hi]
        ch = d_loc // P
        chunk_edges = []
        for t in range(chunks):
            m = ch == t
            chunk_edges.append((s_loc[m], d_loc[m] - t * P))
            counts[c, t] = int(m.sum())
        per_core.append(chunk_edges)

    # tiles per chunk: 1 (self) + ceil(neighbors/128), same across cores
    kmax = [1 + int(-(-counts[:, t].max() // P)) for t in range(chunks)]
    K = max(kmax)
    meta = {"K": K, "kmax": tuple(kmax)}

    pre = []
    for c in range(M):
        src_idx = np.zeros((chunks, P, K), dtype=np.int32)
        S_h = np.zeros((chunks, P, K * P), dtype=ml_dtypes.bfloat16)
        ST_h = np.zeros((chunks, P, K * P), dtype=ml_dtypes.bfloat16)
        for t in range(chunks):
            nt = min(P, shard - t * P)
            sl, dl = per_core[c][t]
            # self tile k=0: slot p -> dst p (only valid rows)
            pp = np.arange(nt)
            S_h[t, pp, pp] = 1.0
            ST_h[t, pp, pp] = 1.0
            # neighbor tiles
            j = np.arange(len(sl))
            p, k = j % P, 1 + j // P
            src_idx[t, p, k] = sl
            S_h[t, p, k * P + dl] = 1.0
            ST_h[t, dl, k * P + p] = 1.0
        pre.append({"src_idx": src_idx, "S_h": S_h, "ST_h": ST_h})
    return pre, meta


# ----------------------------------------------------------------------------
# Kernel builder
# ----------------------------------------------------------------------------

def build(tc, io, cfg, meta):
    from contextlib import ExitStack

    nc = tc.nc
    D, H, L, C = cfg.D, cfg.H, cfg.L, cfg.C
    K, kmax = meta["K"], meta["kmax"]
    shard, chunks = cfg.shard, cfg.chunks

    ctx = ExitStack()
    dram = ctx.enter_context(tc.tile_pool(name="drampool", bufs=1, space="DRAM"))
    consts = ctx.enter_context(tc.tile_pool(name="consts", bufs=1))
    lconsts = ctx.enter_context(tc.tile_pool(name="lconsts", bufs=2))
    nodep = ctx.enter_context(tc.tile_pool(name="nodep", bufs=3))
    idxp = ctx.enter_context(tc.tile_pool(name="idxp", bufs=3))
    edgep = ctx.enter_context(tc.tile_pool(name="edgep", bufs=3))
    smallp = ctx.enter_context(tc.tile_pool(name="smallp", bufs=3))
    ps_o = ctx.enter_context(tc.tile_pool(name="ps_o", bufs=2, space="PSUM"))
    ps_n = ctx.enter_context(tc.tile_pool(name="ps_n", bufs=2, space="PSUM"))
    ps_t = ctx.enter_context(tc.tile_pool(name="ps_t", bufs=1, space="PSUM"))
    ps_me = ctx.enter_context(tc.tile_pool(name="ps_me", bufs=3, space="PSUM"))

    # internal DRAM buffers
    xl_sh = [dram.tile([shard, D], BF16, name=f"xl_sh{l}") for l in range(L)]
    xl_all = [
        dram.tile([cfg.N, D], BF16, name=f"xl_all{l}", addr_space="Shared")
        for l in range(L)
    ]
    xr_dram = [dram.tile([shard, D], F32, name=f"xr_dram{l}") for l in range(L)]
    xst = [dram.tile([shard, D], F32, name=f"xst{l}") for l in range(L - 1)]
    xT = [dram.tile([P, chunks * P], F32, name=f"xT{l}") for l in range(L)]

    ident_sb = consts.tile([P, P], F32, name="ident_sb")
    nc.sync.dma_start(out=ident_sb[:], in_=io["ident"][:, :])

    # prologue: xT[0] = transpose of x_shard
    for t in range(chunks):
        nt = min(P, shard - t * P)
        xq0 = nodep.tile([P, D], F32, name="xq0")
        nc.sync.dma_start(out=xq0[:nt, :], in_=io["x_shard"][t * P : t * P + nt, :])
        psT = ps_t.tile([P, P], F32, name="psT", tag="psT")
        nc.tensor.transpose(
            out=psT[:, :nt], in_=xq0[:nt, :], identity=ident_sb[:nt, :nt]
        )
        sbT = nodep.tile([P, P], F32, name="sbT")
        nc.scalar.activation(out=sbT[:, :nt], in_=psT[:, :nt], func=AF.Copy)
        nc.sync.dma_start(out=xT[0][:, t * P : t * P + nt], in_=sbT[:, :nt])

    for l in range(L):
        # per-layer constants (host-materialized, sync-loaded)
        wl_sb = lconsts.tile([P, D], F32, name="wl_sb")
        nc.sync.dma_start(out=wl_sb[:], in_=io["Wl"][l, :, :])
        wr_sb = lconsts.tile([P, D], F32, name="wr_sb")
        nc.sync.dma_start(out=wr_sb[:], in_=io["Wr"][l, :, :])
        attbK_sb = lconsts.tile([P, K * D], BF16, name="attbK_sb")
        nc.sync.dma_start(out=attbK_sb[:], in_=io["attbK"][l, :, :])
        bc_sb = lconsts.tile([P, D], F32, name="bc_sb")
        nc.sync.dma_start(out=bc_sb[:], in_=io["bc"][l, :, :])
        cvec_sb = lconsts.tile([P, D], F32, name="cvec_sb")
        nc.sync.dma_start(out=cvec_sb[:], in_=io["cvec"][l, :, :])
        gamma_sb = lconsts.tile([P, D], F32, name="gamma_sb")
        nc.sync.dma_start(out=gamma_sb[:], in_=io["gamma"][l, :, :])
        beta_sb = lconsts.tile([P, D], F32, name="beta_sb")
        nc.sync.dma_start(out=beta_sb[:], in_=io["beta"][l, :, :])

        # node phase: xl = x@Wl (bf16), xr = x@Wr + (bl+br) (f32)
        for t in range(chunks):
            nt = min(P, shard - t * P)
            lhsT = nodep.tile([P, P], F32, name="lhsT")
            nc.sync.dma_start(out=lhsT[:, :nt], in_=xT[l][:, t * P : t * P + nt])
            ps_xl = ps_n.tile([P, D], F32, name="ps_xl", tag="ps_n")
            nc.tensor.matmul(
                out=ps_xl[:nt, :], lhsT=lhsT[:, :nt], rhs=wl_sb[:], start=True,
                stop=True,
            )
            xl_o = nodep.tile([P, D], BF16, name="xl_o")
            nc.scalar.activation(out=xl_o[:nt, :], in_=ps_xl[:nt, :], func=AF.Copy)
            nc.sync.dma_start(out=xl_sh[l][t * P : t * P + nt, :], in_=xl_o[:nt, :])

            ps_xr = ps_n.tile([P, D], F32, name="ps_xr", tag="ps_n")
            nc.tensor.matmul(
                out=ps_xr[:nt, :], lhsT=lhsT[:, :nt], rhs=wr_sb[:], start=True,
                stop=True,
            )
            xr_o = nodep.tile([P, D], F32, name="xr_o")
            nc.vector.tensor_tensor(
                out=xr_o[:nt, :], in0=ps_xr[:nt, :], in1=bc_sb[:nt, :], op=ALU.add
            )
            nc.sync.dma_start(out=xr_dram[l][t * P : t * P + nt, :], in_=xr_o[:nt, :])

        # AllGather xl across the 8 cores
        nc.gpsimd.collective_compute(
            "AllGather",
            ALU.bypass,
            replica_groups=[list(range(cfg.M))],
            ins=[xl_sh[l][:, :].opt()],
            outs=[xl_all[l][:, :].opt()],
        )

        # edge phase, one chunk of 128 destinations at a time
        for ch in range(chunks):
            nt = min(P, shard - ch * P)
            rows = slice(ch * P, ch * P + nt)
            kc = kmax[ch]

            srcg_sb = idxp.tile([P, K], I32, name="srcg_sb")
            nc.sync.dma_start(out=srcg_sb[:, :kc], in_=io["src_idx"][ch, :, :kc])
            S_sb = idxp.tile([P, K * P], BF16, name="S_sb")
            nc.sync.dma_start(out=S_sb[:, : kc * P], in_=io["S_h"][ch, :, : kc * P])
            ST_sb = idxp.tile([P, K * P], BF16, name="ST_sb")
            nc.sync.dma_start(
                out=ST_sb[:, : kc * P], in_=io["ST_h"][ch, :, : kc * P]
            )

            # gathered xl rows: k=0 self tile via plain DMA, rest indirect
            g2 = edgep.tile([P, K, D], BF16, name="g2")
            nc.sync.dma_start(out=g2[:nt, 0, :], in_=xl_sh[l][rows, :])
            for k in range(1, kc):
                nc.gpsimd.indirect_dma_start(
                    out=g2[:, k, :],
                    out_offset=None,
                    in_=xl_all[l][:, :],
                    in_offset=IndirectOffsetOnAxis(
                        ap=srcg_sb[:, k : k + 1], axis=0
                    ),
                )

            # xr chunk -> bf16 for the ST segment-broadcast matmuls
            xr_ch = smallp.tile([P, D], F32, name="xr_ch")
            nc.sync.dma_start(out=xr_ch[:nt, :], in_=xr_dram[l][rows, :])
            xr16 = smallp.tile([P, D], BF16, name="xr16")
            nc.scalar.activation(out=xr16[:nt, :], in_=xr_ch[:nt, :], func=AF.Copy)

            # m = xr[dst] (PE broadcast) + g2, grouped 4 k-tiles per PSUM bank
            m16 = edgep.tile([P, K, D], BF16, name="m16")
            for g0 in range(0, kc, 4):
                gn = min(4, kc - g0)
                pm4 = ps_me.tile([P, 4, D], F32, name="pm4", tag="pm4")
                for j in range(gn):
                    nc.tensor.matmul(
                        out=pm4[:, j, :],
                        lhsT=ST_sb[:, (g0 + j) * P : (g0 + j + 1) * P],
                        rhs=xr16[:, :],
                        start=True,
                        stop=True,
                    )
                nc.vector.tensor_tensor(
                    out=m16[:, g0 : g0 + gn, :],
                    in0=pm4[:, :gn, :],
                    in1=g2[:, g0 : g0 + gn, :],
                    op=ALU.add,
                )

            # leaky relu: max(m, 0.2*m)
            lk = edgep.tile([P, K, D], BF16, name="lk")
            nc.vector.tensor_scalar(
                out=lk[:, :kc, :], in0=m16[:, :kc, :], scalar1=NEG_SLOPE,
                scalar2=None, op0=ALU.mult,
            )
            nc.vector.tensor_tensor(
                out=lk[:, :kc, :], in0=lk[:, :kc, :], in1=m16[:, :kc, :],
                op=ALU.max,
            )

            # attention logits + exp
            tt = edgep.tile([P, K * D], BF16, name="tt")
            nc.vector.tensor_tensor(
                out=tt[:, : kc * D],
                in0=lk[:, :kc, :].rearrange("p k d -> p (k d)"),
                in1=attbK_sb[:, : kc * D],
                op=ALU.mult,
            )
            lg = smallp.tile([P, K, H], F32, name="lg")
            nc.vector.reduce_sum(
                out=lg[:, :kc, :],
                in_=tt[:, : kc * D].rearrange("p (k h c) -> p k h c", k=kc, h=H),
                axis=AX.X,
            )
            zee = edgep.tile([P, K, D + H], BF16, name="zee")
            nc.scalar.activation(
                out=zee[:, :kc, D : D + H], in_=lg[:, :kc, :], func=AF.Exp
            )
            nc.vector.tensor_tensor(
                out=zee[:, :kc, 0:D].rearrange("p k (h c) -> p k h c", h=H),
                in0=g2[:, :kc, :].rearrange("p k (h c) -> p k h c", h=H),
                in1=zee[:, :kc, D : D + H].unsqueeze(3).to_broadcast(
                    [P, kc, H, C]
                ),
                op=ALU.mult,
            )

            # segment sums on PE: po[dst, 0:D] = sum ee*g ; po[dst, D:] = denom
            po = ps_o.tile([P, D + H], F32, name="po")
            for k in range(kc):
                nc.tensor.matmul(
                    out=po[:, :],
                    lhsT=S_sb[:, k * P : (k + 1) * P],
                    rhs=zee[:, k, :],
                    start=(k == 0),
                    stop=(k == kc - 1),
                )

            dn = smallp.tile([P, H], F32, name="dn")
            nc.vector.tensor_scalar(
                out=dn[:, :], in0=po[:, D : D + H], scalar1=DENOM_EPS,
                scalar2=None, op0=ALU.add,
            )
            rd = smallp.tile([P, H], F32, name="rd")
            nc.vector.reciprocal(out=rd[:, :], in_=dn[:, :])

            onrm = smallp.tile([P, D], F32, name="onrm")
            nc.vector.tensor_tensor(
                out=onrm[:, :].rearrange("p (h c) -> p h c", h=H),
                in0=po[:, 0:D].rearrange("p (h c) -> p h c", h=H),
                in1=rd[:, :].unsqueeze(2).to_broadcast([P, H, C]),
                op=ALU.mult,
            )

            # h = onrm + (bl + gat_bias); then residual + LN
            xq = smallp.tile([P, D], F32, name="xq")
            if l == 0:
                nc.sync.dma_start(out=xq[:nt, :], in_=io["x_shard"][rows, :])
            else:
                nc.sync.dma_start(out=xq[:nt, :], in_=xst[l - 1][rows, :])

            t2 = smallp.tile([P, D], F32, name="t2")
            nc.vector.tensor_tensor(
                out=t2[:nt, :], in0=onrm[:nt, :], in1=cvec_sb[:nt, :], op=ALU.add
            )
            t3 = smallp.tile([P, D], F32, name="t3")
            nc.vector.tensor_tensor(
                out=t3[:nt, :], in0=t2[:nt, :], in1=xq[:nt, :], op=ALU.add
            )

            st6 = smallp.tile([P, 6], F32, name="st6")
            nc.vector.bn_stats(out=st6[:nt, :], in_=t3[:nt, :])
            mv = smallp.tile([P, 2], F32, name="mv")
            nc.vector.bn_aggr(out=mv[:nt, :], in_=st6[:nt, :])
            veps = smallp.tile([P, 1], F32, name="veps")
            nc.vector.tensor_scalar(
                out=veps[:nt, :], in0=mv[:nt, 1:2], scalar1=LN_EPS, scalar2=None,
                op0=ALU.add,
            )
            sd = smallp.tile([P, 1], F32, name="sd")
            nc.scalar.activation(out=sd[:nt, :], in_=veps[:nt, :], func=AF.Sqrt)
            rstd = smallp.tile([P, 1], F32, name="rstd")
            nc.vector.reciprocal(out=rstd[:nt, :], in_=sd[:nt, :])

            y1 = smallp.tile([P, D], F32, name="y1")
            nc.vector.tensor_scalar(
                out=y1[:nt, :], in0=t3[:nt, :], scalar1=mv[:nt, 0:1],
                scalar2=rstd[:nt, :], op0=ALU.subtract, op1=ALU.mult,
            )
            y2 = smallp.tile([P, D], F32, name="y2")
            nc.vector.tensor_tensor(
                out=y2[:nt, :], in0=y1[:nt, :], in1=gamma_sb[:nt, :], op=ALU.mult
            )
            y3 = smallp.tile([P, D], F32, name="y3")
            nc.vector.tensor_tensor(
                out=y3[:nt, :], in0=y2[:nt, :], in1=beta_sb[:nt, :], op=ALU.add
            )

            if l < L - 1:
                xo = smallp.tile([P, D], F32, name="xo")
                nc.scalar.activation(out=xo[:nt, :], in_=y3[:nt, :], func=AF.Relu)
                nc.sync.dma_start(out=xst[l][rows, :], in_=xo[:nt, :])
                psT2 = ps_t.tile([P, P], F32, name="psT2", tag="psT")
                nc.tensor.transpose(
                    out=psT2[:, :nt], in_=xo[:nt, :], identity=ident_sb[:nt, :nt]
                )
                sbT2 = smallp.tile([P, P], F32, name="sbT2")
                nc.scalar.activation(out=sbT2[:, :nt], in_=psT2[:, :nt], func=AF.Copy)
                nc.sync.dma_start(
                    out=xT[l + 1][:, ch * P : ch * P + nt], in_=sbT2[:, :nt]
                )
            else:
                nc.sync.dma_start(out=io["y"][rows, :], in_=y3[:nt, :])

    ctx.close()


# ----------------------------------------------------------------------------
# host-side inputs
# ----------------------------------------------------------------------------

def make_host_inputs(inputs, cfg, K):
    L, D, H, C = cfg.L, cfg.D, cfg.H, cfg.C
    Wl = np.asarray(inputs["Wl"], np.float32)
    Wr = np.asarray(inputs["Wr"], np.float32)
    bl = np.asarray(inputs["bl"], np.float32)
    br = np.asarray(inputs["br"], np.float32)
    att = np.asarray(inputs["att"], np.float32)
    gat_bias = np.asarray(inputs["bias"], np.float32)
    gamma = np.asarray(inputs["gamma"], np.float32)
    beta = np.asarray(inputs["beta"], np.float32)

    def bcast(v):  # [L, D] -> [L, P, D] replicated rows
        return np.ascontiguousarray(
            np.broadcast_to(v[:, None, :], (L, P, v.shape[-1]))
        ).astype(np.float32)

    att_flat = att.reshape(L, 1, H * C)
    attbK = np.ascontiguousarray(
        np.broadcast_to(np.tile(att_flat, (1, 1, K)), (L, P, K * D))
    ).astype(ml_dtypes.bfloat16)
    return {
        "Wl": Wl,
        "Wr": Wr,
        "attbK": attbK,
        "bc": bcast(bl + br),
        "cvec": bcast(bl + gat_bias),
        "gamma": bcast(gamma),
        "beta": bcast(beta),
        "ident": np.eye(P, dtype=np.float32),
    }


def make_in_maps(inputs, pre, cfg, K):
    x = np.asarray(inputs["fine_poi_x"], np.float32)
    shared = make_host_inputs(inputs, cfg, K)
    in_maps = []
    for c in range(cfg.M):
        m = dict(shared)
        m["x_shard"] = np.ascontiguousarray(
            x[c * cfg.shard : (c + 1) * cfg.shard]
        )
        for k in ("src_idx", "S_h", "ST_h"):
            m[k] = pre[c][k]
        in_maps.append(m)
    return in_maps


# ----------------------------------------------------------------------------
# program assembly + execution
# ----------------------------------------------------------------------------

_CACHE = {}


def _build_program(cfg, meta):
    K = meta["K"]
    key = (cfg.N, cfg.D, cfg.H, cfg.L, cfg.M, K, meta["kmax"])
    if key in _CACHE:
        return _CACHE[key]
    nc = bacc.Bacc(
        "TRN2", target_bir_lowering=False, debug=False, num_devices=cfg.M
    )
    io = {}
    io["x_shard"] = nc.dram_tensor(
        "x_shard", [cfg.shard, cfg.D], F32, kind="ExternalInput"
    ).ap()
    io["src_idx"] = nc.dram_tensor(
        "src_idx", [cfg.chunks, P, K], I32, kind="ExternalInput"
    ).ap()
    io["S_h"] = nc.dram_tensor(
        "S_h", [cfg.chunks, P, K * P], BF16, kind="ExternalInput"
    ).ap()
    io["ST_h"] = nc.dram_tensor(
        "ST_h", [cfg.chunks, P, K * P], BF16, kind="ExternalInput"
    ).ap()
    io["attbK"] = nc.dram_tensor(
        "attbK", [cfg.L, P, K * cfg.D], BF16, kind="ExternalInput"
    ).ap()
    io["Wl"] = nc.dram_tensor(
        "Wl", [cfg.L, cfg.D, cfg.D], F32, kind="ExternalInput"
    ).ap()
    io["Wr"] = nc.dram_tensor(
        "Wr", [cfg.L, cfg.D, cfg.D], F32, kind="ExternalInput"
    ).ap()
    for nm in ["bc", "cvec", "gamma", "beta"]:
        io[nm] = nc.dram_tensor(
            nm, [cfg.L, P, cfg.D], F32, kind="ExternalInput"
        ).ap()
    io["ident"] = nc.dram_tensor("ident", [P, P], F32, kind="ExternalInput").ap()
    io["y"] = nc.dram_tensor(
        "y", [cfg.shard, cfg.D], F32, kind="ExternalOutput"
    ).ap()

    with tile.TileContext(nc) as tc:
        build(tc, io, cfg, meta)
    nc.compile()
    _CACHE[key] = nc
    return nc


def kernel(**inputs):
    from concourse import bass_utils

    cfg = Cfg()
    pre, meta = preprocess(inputs["edge_index"], cfg)
    nc = _build_program(cfg, meta)
    in_maps = make_in_maps(inputs, pre, cfg, meta["K"])
    res = bass_utils.run_bass_kernel_spmd(
        nc, in_maps, core_ids=list(range(cfg.M))
    )
    out = np.concatenate([res.results[c]["y"] for c in range(cfg.M)], axis=0)
    return out.astype(np.float32)
